# revision 2
# baseline (speedup 1.0000x reference)
"""Trainium2 Bass kernel for nn_Gate_Net (Toeplitz + hard-sigmoid prob + cumprod gate).

Reference computation (per document row of 1024 scores):
  s = doc[1:-1]                      # n = 1022
  score_hat[r, j] = s[j-1-r] if j-1-r >= 0 else 0      # [1021, 1022]
  p[r, j] = clamp(10*(score_hat - s[j]) + 1, 0, 1)      # hard branch, res=0.1
  fwd = cumprod(p, axis=0); bwd = same with s reversed
  out = stack([fwd, bwd]) per doc -> full [32, 2, 1021, 1022] f32

Device algorithm (per doc-direction, column-major):
  Column j of the output is a cumprod over m of factors
    q(j, m) = clamp(10*s[j-1-m] + c_j, 0, 1), c_j = 1 - 10*s[j]   (m < j)
    q(j, m) = clamp(c_j, 0, 1)                                     (m >= j)
  We materialize, with ONE diagonal-AP DMA per doc-dir, the sheared tile
    B[p, t] = arr[127 - p + t]   where arr = [0, reversed(10*s), 0-pad]
  so that every 128-column block's factor matrix is a plain uniform-offset
  slice of B (zeros beyond the data edge give exactly the boundary factor).
  Then: tensor_scalar(add c_j, min 1) -> Relu -> tensor_tensor_scan
  (cumprod along the free axis) -> PE transpose -> contiguous row stores.

Sharding: pure data parallel, 8 doc-dirs per core (4 docs x fwd/bwd).
"""
import numpy as np

import concourse.bass as bass
import concourse.bacc as bacc
import concourse.tile as tile
from concourse import mybir
from concourse import bass_utils
from concourse.masks import make_identity

P = 128
N = 1022          # columns j per doc-dir
ROWS = N - 1      # 1021 output rows
NB = 8            # column blocks (last has 126 valid columns)
MB = 8            # row blocks (last has 125 valid rows)
ARRW = 1152       # padded diag-source array width
BW = 1024         # sheared tile width

_NC_CACHE: dict = {}


def build_nc(n_dd: int = 8):
    """Build the single-core Bass program processing n_dd doc-dirs."""
    nc = bacc.Bacc("TRN2", target_bir_lowering=False, debug=False, num_devices=8)
    arr = nc.dram_tensor("arr", [n_dd, ARRW], mybir.dt.float32, kind="ExternalInput")
    cc = nc.dram_tensor("cc", [n_dd, P, 16], mybir.dt.float32, kind="ExternalInput")
    out = nc.dram_tensor("out", [n_dd, ROWS, N], mybir.dt.float32, kind="ExternalOutput")

    add_op = mybir.AluOpType.add
    min_op = mybir.AluOpType.min
    mult_op = mybir.AluOpType.mult
    relu = mybir.ActivationFunctionType.Relu

    with tile.TileContext(nc) as tc:
        with (
            tc.tile_pool(name="consts", bufs=1) as consts,
            tc.tile_pool(name="bsrc", bufs=2) as bsrc_pool,
            tc.tile_pool(name="qpool", bufs=2) as qpool,
            tc.tile_pool(name="rpool", bufs=2) as rpool,
            tc.tile_pool(name="cpool", bufs=2) as cpool,
            tc.tile_pool(name="outp", bufs=2) as outp,
            tc.tile_pool(name="psum", bufs=8, space="PSUM") as psum,
        ):
            # flip permutation: flip[k, n] = 1 iff k + n == P-1.  Used as the
            # matmul rhs in the PE transpose so that the partition-flipped
            # column order (p <-> j = jb*128 + 127 - p) comes out natural.
            flip = consts.tile([P, P], mybir.dt.float32)
            nc.gpsimd.memset(flip[:], 0.0)
            nc.gpsimd.affine_select(
                out=flip[:], in_=flip[:],
                compare_op=mybir.AluOpType.not_equal, fill=1.0,
                base=-(P - 1), pattern=[[1, P]], channel_multiplier=1,
            )
            zeros = consts.tile([P, ROWS], mybir.dt.float32)
            nc.vector.memset(zeros[:], 0.0)

            for dd in range(n_dd):
                B = bsrc_pool.tile([P, BW], mybir.dt.float32, tag="B")
                diag_src = bass.AP(
                    tensor=arr, offset=dd * ARRW, ap=[[1, P], [1, BW]]
                )
                nc.sync.dma_start(out=B[:], in_=diag_src)

                csb = cpool.tile([P, 16], mybir.dt.float32, tag="csb")
                nc.sync.dma_start(out=csb[:], in_=cc[dd, :, :])

                outsb = [
                    outp.tile([P, N], mybir.dt.float32, tag=f"o{mb}", name=f"osb{mb}")
                    for mb in range(MB)
                ]

                for jb in range(NB):
                    W = min(jb * 128 + 128, ROWS)
                    y = 896 - jb * 128
                    Q = qpool.tile([P, ROWS], mybir.dt.float32, tag="Q", name="Q")
                    # q_pre = min(B_slice + c_j, 1); factor = relu(q_pre)
                    nc.vector.tensor_scalar(
                        out=Q[:, 0:W],
                        in0=B[:, y:y + W],
                        scalar1=csb[:, jb:jb + 1],
                        scalar2=1.0,
                        op0=add_op,
                        op1=min_op,
                    )
                    nc.scalar.activation(
                        out=Q[:, 0:W], in_=Q[:, 0:W], func=relu, bias=0.0, scale=1.0
                    )
                    if W < ROWS:
                        # tail factors: clamp(c_j, 0, 1) = relu(min(c_j, 1))
                        nc.scalar.activation(
                            out=Q[:, W:ROWS],
                            in_=B[:, 0:ROWS - W],
                            func=relu,
                            bias=csb[:, 8 + jb:8 + jb + 1],
                            scale=0.0,
                        )
                    R = rpool.tile([P, ROWS], mybir.dt.float32, tag="R", name="R")
                    nc.vector.tensor_tensor_scan(
                        out=R[:],
                        data0=Q[:],
                        data1=zeros[:],
                        initial=1.0,
                        op0=mult_op,
                        op1=add_op,
                    )
                    cols = 126 if jb == NB - 1 else 128
                    for mb in range(MB):
                        chunk = 125 if mb == MB - 1 else 128
                        pt = psum.tile([P, P], mybir.dt.float32, tag="pt", name="pt")
                        nc.tensor.transpose(
                            pt[:chunk, :], R[:, mb * 128:mb * 128 + chunk], flip[:]
                        )
                        dst = outsb[mb][:chunk, jb * 128:jb * 128 + cols]
                        if mb % 2 == 0:
                            nc.scalar.copy(out=dst, in_=pt[:chunk, 0:cols])
                        else:
                            nc.vector.tensor_copy(dst, pt[:chunk, 0:cols])

                for mb in range(MB):
                    chunk = 125 if mb == MB - 1 else 128
                    nc.sync.dma_start(
                        out=out[dd, mb * 128:mb * 128 + chunk, :],
                        in_=outsb[mb][:chunk, :],
                    )
    nc.compile()
    return nc


def get_nc(n_dd: int = 8):
    if n_dd not in _NC_CACHE:
        _NC_CACHE[n_dd] = build_nc(n_dd)
    return _NC_CACHE[n_dd]


def make_core_inputs(docs_core: np.ndarray) -> dict:
    """docs_core: [n_docs, 1024] f32 -> in_map with arr/cc for n_docs*2 doc-dirs."""
    n_docs = docs_core.shape[0]
    n_dd = n_docs * 2
    arr = np.zeros((n_dd, ARRW), np.float32)
    cc = np.zeros((n_dd, P, 16), np.float32)
    for dl in range(n_docs):
        s = docs_core[dl, 1:-1].astype(np.float32)  # 1022
        for t in range(2):
            v = s if t == 0 else s[::-1]
            dd = dl * 2 + t
            v10 = (np.float32(10.0) * v).astype(np.float32)
            arr[dd, 1:1 + N] = v10[::-1]
            cvals = (np.float32(1.0) - v10).astype(np.float32)
            # partition p holds column j = jb*128 + (127 - p)
            for jb in range(NB):
                seg = cvals[jb * 128: jb * 128 + 128]
                cseg = np.zeros(P, np.float32)
                cseg[P - len(seg):] = seg[::-1]
                cc[dd, :, jb] = cseg
                cc[dd, :, 8 + jb] = np.minimum(cseg, np.float32(1.0))
    return {"arr": arr, "cc": cc}


def make_in_maps(score: np.ndarray, score_idx: np.ndarray):
    """Helper for the test harness: full inputs -> per-core in_maps."""
    score = np.asarray(score, dtype=np.float32)
    docs = score[np.asarray(score_idx)]
    n_cores = 8
    dpc = docs.shape[0] // n_cores
    in_maps = [make_core_inputs(docs[c * dpc:(c + 1) * dpc]) for c in range(n_cores)]
    return in_maps, None


def kernel(score: np.ndarray, score_idx: np.ndarray) -> np.ndarray:
    score = np.asarray(score, dtype=np.float32)
    score_idx = np.asarray(score_idx)
    docs = score[score_idx]  # [B, L] gather
    Bn, L = docs.shape       # 32, 1024
    n_cores = 8
    docs_per_core = Bn // n_cores  # 4

    in_maps = [
        make_core_inputs(docs[c * docs_per_core:(c + 1) * docs_per_core])
        for c in range(n_cores)
    ]
    nc = get_nc(docs_per_core * 2)
    res = bass_utils.run_bass_kernel_spmd(nc, in_maps, core_ids=list(range(n_cores)))
    full = np.empty((Bn, 2, ROWS, N), np.float32)
    for c in range(n_cores):
        o = np.asarray(res.results[c]["out"]).reshape(docs_per_core * 2, ROWS, N)
        for dl in range(docs_per_core):
            for t in range(2):
                full[c * docs_per_core + dl, t] = o[dl * 2 + t]
    return full



# revision 3
# speedup vs baseline: 2.2672x; 2.2672x over previous
"""Trainium2 Bass kernel for nn_Gate_Net (Toeplitz + hard-sigmoid prob + cumprod gate).

Reference computation (per document row of 1024 scores):
  s = doc[1:-1]                      # n = 1022
  score_hat[r, j] = s[j-1-r] if j-1-r >= 0 else 0      # [1021, 1022]
  p[r, j] = clamp(10*(score_hat - s[j]) + 1, 0, 1)      # hard branch, res=0.1
  fwd = cumprod(p, axis=0); bwd = same with s reversed
  out = stack([fwd, bwd]) per doc -> full [32, 2, 1021, 1022] f32

Device algorithm (per doc-direction, column-major):
  Column j of the output is a cumprod over m of factors
    q(j, m) = clamp(10*s[j-1-m] + c_j, 0, 1), c_j = 1 - 10*s[j]   (m < j)
    q(j, m) = clamp(c_j, 0, 1)                                     (m >= j)
  We materialize, with ONE diagonal-AP DMA per doc-dir, the sheared tile
    B[p, t] = arr[p + t]   where arr = [0, reversed(10*s), 0-pad]
  so that every 128-column block's factor matrix is a plain uniform-offset
  slice of B (zeros beyond the data edge give exactly the boundary factor).
  Then: tensor_scalar(add c_j, min 1) -> tensor_tensor_scan with
  (op0=mult, op1=max vs 0), which both applies the lower clamp (running
  state is >= 0, so max(q*state, 0) == clamp(q,0,1)*state) and performs
  the cumprod along the free axis.  The scan result is written in bf16
  COLUMN-major ([j, r]) straight to DRAM -- no on-device transpose; the
  host gather step undoes the (block, reversed-partition) permutation
  and upcasts to f32.

Sharding: pure data parallel, 8 doc-dirs per core (4 docs x fwd/bwd).
"""
import numpy as np

import concourse.bass as bass
import concourse.bacc as bacc
import concourse.tile as tile
from concourse import mybir
from concourse import bass_utils

P = 128
N = 1022          # columns j per doc-dir
ROWS = N - 1      # 1021 output rows
NB = 8            # column blocks (last has 126 valid columns)
ARRW = 1152       # padded diag-source array width
BW = 1024         # sheared tile width

_NC_CACHE: dict = {}


def build_nc(n_dd: int = 8):
    """Build the single-core Bass program processing n_dd doc-dirs."""
    nc = bacc.Bacc("TRN2", target_bir_lowering=False, debug=False, num_devices=8)
    arr = nc.dram_tensor("arr", [n_dd, ARRW], mybir.dt.float32, kind="ExternalInput")
    cc = nc.dram_tensor("cc", [n_dd, P, 16], mybir.dt.float32, kind="ExternalInput")
    out = nc.dram_tensor(
        "out", [n_dd, NB, P, ROWS], mybir.dt.bfloat16, kind="ExternalOutput"
    )

    add_op = mybir.AluOpType.add
    min_op = mybir.AluOpType.min
    mult_op = mybir.AluOpType.mult
    max_op = mybir.AluOpType.max
    relu = mybir.ActivationFunctionType.Relu

    with tile.TileContext(nc) as tc:
        with (
            tc.tile_pool(name="consts", bufs=1) as consts,
            tc.tile_pool(name="bsrc", bufs=2) as bsrc_pool,
            tc.tile_pool(name="qpool", bufs=3) as qpool,
            tc.tile_pool(name="rpool", bufs=3) as rpool,
            tc.tile_pool(name="cpool", bufs=2) as cpool,
        ):
            zeros = consts.tile([P, ROWS], mybir.dt.float32)
            nc.vector.memset(zeros[:], 0.0)

            for dd in range(n_dd):
                B = bsrc_pool.tile([P, BW], mybir.dt.float32, tag="B")
                diag_src = bass.AP(
                    tensor=arr, offset=dd * ARRW, ap=[[1, P], [1, BW]]
                )
                nc.sync.dma_start(out=B[:], in_=diag_src)

                csb = cpool.tile([P, 16], mybir.dt.float32, tag="csb")
                nc.sync.dma_start(out=csb[:], in_=cc[dd, :, :])

                for jb in range(NB):
                    W = min(jb * 128 + 128, ROWS)
                    y = 896 - jb * 128
                    Q = qpool.tile([P, ROWS], mybir.dt.float32, tag="Q", name="Q")
                    # q_pre = min(B_slice + c_j, 1); the lower clamp happens
                    # inside the scan (op1 = max vs 0).
                    nc.vector.tensor_scalar(
                        out=Q[:, 0:W],
                        in0=B[:, y:y + W],
                        scalar1=csb[:, jb:jb + 1],
                        scalar2=1.0,
                        op0=add_op,
                        op1=min_op,
                    )
                    if W < ROWS:
                        # tail factors: constant clamp(c_j, 0, 1) broadcast
                        # (relu of the host-precomputed min(c_j, 1)).
                        nc.scalar.activation(
                            out=Q[:, W:ROWS],
                            in_=B[:, 0:ROWS - W],
                            func=relu,
                            bias=csb[:, 8 + jb:8 + jb + 1],
                            scale=0.0,
                        )
                    R = rpool.tile([P, ROWS], mybir.dt.bfloat16, tag="R", name="R")
                    nc.vector.tensor_tensor_scan(
                        out=R[:],
                        data0=Q[:],
                        data1=zeros[:],
                        initial=1.0,
                        op0=mult_op,
                        op1=max_op,
                    )
                    nc.sync.dma_start(out=out[dd, jb, :, :], in_=R[:])
    nc.compile()
    return nc


def get_nc(n_dd: int = 8):
    if n_dd not in _NC_CACHE:
        _NC_CACHE[n_dd] = build_nc(n_dd)
    return _NC_CACHE[n_dd]


def make_core_inputs(docs_core: np.ndarray) -> dict:
    """docs_core: [n_docs, 1024] f32 -> in_map with arr/cc for n_docs*2 doc-dirs."""
    n_docs = docs_core.shape[0]
    n_dd = n_docs * 2
    arr = np.zeros((n_dd, ARRW), np.float32)
    cc = np.zeros((n_dd, P, 16), np.float32)
    for dl in range(n_docs):
        s = docs_core[dl, 1:-1].astype(np.float32)  # 1022
        for t in range(2):
            v = s if t == 0 else s[::-1]
            dd = dl * 2 + t
            v10 = (np.float32(10.0) * v).astype(np.float32)
            arr[dd, 1:1 + N] = v10[::-1]
            cvals = (np.float32(1.0) - v10).astype(np.float32)
            # partition p holds column j = jb*128 + (127 - p)
            for jb in range(NB):
                seg = cvals[jb * 128: jb * 128 + 128]
                cseg = np.zeros(P, np.float32)
                cseg[P - len(seg):] = seg[::-1]
                cc[dd, :, jb] = cseg
                cc[dd, :, 8 + jb] = np.minimum(cseg, np.float32(1.0))
    return {"arr": arr, "cc": cc}


def make_in_maps(score: np.ndarray, score_idx: np.ndarray):
    """Helper for the test harness: full inputs -> per-core in_maps."""
    score = np.asarray(score, dtype=np.float32)
    docs = score[np.asarray(score_idx)]
    n_cores = 8
    dpc = docs.shape[0] // n_cores
    in_maps = [make_core_inputs(docs[c * dpc:(c + 1) * dpc]) for c in range(n_cores)]
    return in_maps, None


def kernel(score: np.ndarray, score_idx: np.ndarray) -> np.ndarray:
    score = np.asarray(score, dtype=np.float32)
    score_idx = np.asarray(score_idx)
    docs = score[score_idx]  # [B, L] gather
    Bn, L = docs.shape       # 32, 1024
    n_cores = 8
    docs_per_core = Bn // n_cores  # 4

    in_maps = [
        make_core_inputs(docs[c * docs_per_core:(c + 1) * docs_per_core])
        for c in range(n_cores)
    ]
    nc = get_nc(docs_per_core * 2)
    res = bass_utils.run_bass_kernel_spmd(nc, in_maps, core_ids=list(range(n_cores)))
    full = np.empty((Bn, 2, ROWS, N), np.float32)
    for c in range(n_cores):
        o = np.asarray(res.results[c]["out"])  # [n_dd, NB, P, ROWS] bf16
        o32 = o.astype(np.float32)
        # device R[p, r] holds out[r, j] for j = jb*128 + 127 - p:
        # reverse partitions so index n = 127 - p is the in-block column,
        # then [dd, jb, n, r] -> [dd, r, jb*128 + n].
        o32 = o32[:, :, ::-1, :]
        o32 = np.transpose(o32, (0, 3, 1, 2)).reshape(
            docs_per_core * 2, ROWS, NB * P
        )[:, :, :N]
        for dl in range(docs_per_core):
            for t in range(2):
                full[c * docs_per_core + dl, t] = o32[dl * 2 + t]
    return full
